# revision 1
# baseline (speedup 1.0000x reference)
"""BNAF forward (B=2048, D=8, H=512, 4 masked layers) on 8 TRN2 NeuronCores.

Strategy
--------
Pure data parallel: batch is split 256/core; the small weights are replicated.

Math: the BNAF log-det recursion collapses in exp space.  For each masked
linear layer, exp(logdet diag blocks) == the diag blocks of the normalized
weight w itself, and for tanh, exp(logdet) == 1 - h^2.  So the whole
log-sum-exp flow is a chain of *positive* block-diagonal matmuls with one
log() at the very end.  The per-output norm scale s = exp(logg)/||v|| is
folded into the NEXT layer's flow weights (G-hat formulation), so the flow
elementwise path is a single un-scaled [128, 1024] op per layer.

Device layout: activations transposed ([feature, batch]); h_l and G_l are
single [128, 4*256] tiles (4 o-chunks side by side in the free dim).  The
whole weight matrix DMAs in one shot into the f32r vt tile (the upper
triangle lands as junk but is never read); diag squares are exp'd in place.
Norms use ones-stationary window matmuls (no 128-col LDWEIGHTS) and tiny
K=1 transposes to columnize.  Matmuls are fp32r single-pass.  ACT uses only
{Exp, Tanh, Square, Copy} (one table) + one final Ln; sqrt is a DVE Newton
rsqrt.
"""

import os
import numpy as np

TRACE = False          # set by test.py for profiling runs
LAST_RESULTS = None    # BassKernelResults stash for test.py

_CACHE = {}

P = 128
BC = 256          # batch per core
H = 512
NCORE = 8
MAGIC = 0x5f3759df

# smalls layout: first the exp-batch block (exp'd in one ACT op), then rest
_SM = {}
_off = 0
for _name, _w in [("w1dg", 4), ("w4dg", 4), ("lg1", 4), ("lg2", 4),
                  ("lg3", 4), ("lg4c", 1),                      # <- exp block
                  ("b4c", 1), ("b1", 4), ("b2", 4), ("b3", 4),
                  ("ident", 128), ("w1n", 32), ("w4t", 32),
                  ("md1n", 32), ("mo1n", 32), ("md4t", 32), ("mo4t", 32)]:
    _SM[_name] = (_off, _off + _w)
    _off += _w
SMALL_W = _off
EXPW = _SM["lg4c"][1]          # width of the exp block (21)


def _build():
    import concourse.bacc as bacc
    import concourse.mybir as mybir
    import concourse.tile as tile
    from contextlib import ExitStack

    f32 = mybir.dt.float32
    f32r = mybir.dt.float32r
    u32 = mybir.dt.uint32
    bf16 = mybir.dt.bfloat16
    fp16 = mybir.dt.float16
    E = mybir.ActivationFunctionType
    ALU = mybir.AluOpType

    nc = bacc.Bacc("TRN2", target_bir_lowering=False, debug=False,
                   enable_asserts=False, num_devices=NCORE)

    t = {}
    t["xT"] = nc.dram_tensor("xT", (8, BC), f32, kind="ExternalInput").ap()
    t["w2T"] = nc.dram_tensor("w2T", (H, H), f32, kind="ExternalInput").ap()
    t["w3T"] = nc.dram_tensor("w3T", (H, H), f32, kind="ExternalInput").ap()
    t["smalls"] = nc.dram_tensor("smalls", (P, SMALL_W), f32, kind="ExternalInput").ap()
    t["hT_out"] = nc.dram_tensor("hT_out", (8, BC), f32, kind="ExternalOutput").ap()
    t["sldT_out"] = nc.dram_tensor("sldT_out", (8, BC), f32, kind="ExternalOutput").ap()

    def mm(out, lhsT, rhs, **kw):
        nc.tensor.matmul(out, lhsT, rhs, **kw)

    with tile.TileContext(nc) as tc, ExitStack() as ctx:
        wgt = ctx.enter_context(tc.tile_pool(name="wgt", bufs=1))
        scr = ctx.enter_context(tc.tile_pool(name="scr", bufs=3))
        stg = ctx.enter_context(tc.tile_pool(name="stg", bufs=2))
        pz = ctx.enter_context(tc.tile_pool(name="pz", bufs=3, space="PSUM"))
        pwarm = ctx.enter_context(tc.tile_pool(name="pwarm", bufs=1, space="PSUM"))
        pf = ctx.enter_context(tc.tile_pool(name="pf", bufs=1, space="PSUM"))
        pn = ctx.enter_context(tc.tile_pool(name="pn", bufs=2, space="PSUM"))

        act = nc.scalar.activation
        cp = nc.vector.tensor_copy
        ts = nc.vector.tensor_scalar
        stt = nc.vector.scalar_tensor_tensor
        mul = nc.vector.tensor_mul
        tt = nc.vector.tensor_tensor

        # ---- input DMAs: only 4, smallest first (HWDGE FIFO) ------------
        smalls = wgt.tile([P, SMALL_W], f32, name="smalls_t", tag="smalls_t")
        nc.sync.dma_start(smalls, t["smalls"])
        xT = wgt.tile([8, BC], fp16, name="xT_t", tag="xT_t")
        nc.gpsimd.dma_start(xT, t["xT"])
        # per-chunk window DMAs straight into vt (pipelined arrival);
        # only cols >= 128k of chunk k are ever read
        vt_t = {}
        for l in (2, 3):
            vt_t[l] = wgt.tile([P, 4 * H], fp16, name=f"vt{l}", tag=f"vt{l}")
        for k in range(4):
            for l in (2, 3):
                F = H * k
                nc.gpsimd.dma_start(vt_t[l][:, F + P * k:F + H],
                                    t[f"w{l}T"][P * k:P * k + P, P * k:H])

        def sm(name):
            a, b = _SM[name]
            return smalls[:, a:b]

        ident = sm("ident")
        w1n = sm("w1n")
        w4t = sm("w4t")
        lg4 = smalls[0:8, _SM["lg4c"][0]:_SM["lg4c"][1]]
        b4 = smalls[0:8, _SM["b4c"][0]:_SM["b4c"][1]]

        with tc.high_priority():
            # one batched exp over [w1dg | w4dg | lg1 | lg2 | lg3 | lg4c]
            esm = wgt.tile([P, EXPW], f32, name="esm", tag="esm")
            act(esm, smalls[:, 0:EXPW], E.Exp)
            # batched exps of the tiny layer-1/4 weight mats
            e1n = wgt.tile([P, 32], f32, name="e1n", tag="e1n")
            act(e1n, w1n, E.Exp)
            e4t = wgt.tile([P, 32], f32, name="e4t", tag="e4t")
            act(e4t, w4t, E.Exp)
        e1d = esm[:, _SM["w1dg"][0]:_SM["w1dg"][1]]
        e4d = esm[:, _SM["w4dg"][0]:_SM["w4dg"][1]]
        eg = {1: esm[:, _SM["lg1"][0]:_SM["lg1"][1]],
              2: esm[:, _SM["lg2"][0]:_SM["lg2"][1]],
              3: esm[:, _SM["lg3"][0]:_SM["lg3"][1]]}
        eg4 = esm[0:8, _SM["lg4c"][0]:_SM["lg4c"][1]]

        ones2f = wgt.tile([P, 2], f32, name="ones2f", tag="ones2f")
        z64 = wgt.tile([64, 64], f32, name="z64", tag="z64")
        magict = wgt.tile([P, 4], u32, name="magict", tag="magict")
        with tc.high_priority():
            nc.gpsimd.memset(ones2f, 1.0)
            nc.gpsimd.memset(z64, 0.0)
            nc.gpsimd.memset(magict, MAGIC)
        ones2 = wgt.tile([P, 2], fp16, name="ones2", tag="ones2")
        with tc.high_priority():
            cp(ones2, ones2f)
        # PE warm-up: ~16 dummy fp16 matmuls on zeros keep the PE busy through
        # the DMA prologue so the HAM un-throttles (1.2 -> 2.4 GHz) before the
        # real matmul stream starts
        wz = wgt.tile([P, H], fp16, name="wz", tag="wz")
        pw = pwarm.tile([2, H], f32, name="pw", tag="pw")
        with tc.high_priority():
            nc.vector.memset(wz[:, 0:2], 0.0)
            nc.vector.memset(wz[:, 2:H], 0.0)
            for _ in range(16):
                mm(pw[:, 0:510], wz[:, 0:2], wz[:, 2:H], skip_group_check=True)

        # s_cols = eg * rsqrt(norm2): DVE-only Newton rsqrt
        def make_scale(n2_ap, eg_ap, shape, nm):
            pr = shape[0]
            n2s = scr.tile(list(shape), f32, name=f"n2s_{nm}", tag="sc_n2s")
            cp(n2s, n2_ap)
            shf = scr.tile(list(shape), u32, name=f"shf_{nm}", tag="sc_shf")
            ts(shf, n2s.bitcast(u32), 1, None, op0=ALU.arith_shift_right)
            y0 = scr.tile(list(shape), u32, name=f"y0_{nm}", tag="sc_y0")
            stt(y0, magict[:pr, :shape[1]], 0, shf, op0=ALU.bypass, op1=ALU.subtract)
            y = y0.bitcast(f32)
            t1 = scr.tile(list(shape), f32, name=f"t1_{nm}", tag="sc_t1")
            t2 = scr.tile(list(shape), f32, name=f"t2_{nm}", tag="sc_t2")
            for it in range(2):         # two Newton steps: y *= 1.5 - 0.5*n2*y*y
                mul(t1, y, y)
                mul(t2, t1, n2s)
                ts(t1, t2, -0.5, 1.5, op0=ALU.mult, op1=ALU.add)
                yn = scr.tile(list(shape), f32, name=f"yn{it}_{nm}", tag=f"sc_yn{it}")
                mul(yn, y, t1)
                y = yn
            s = wgt.tile(list(shape), f32, name=f"s_{nm}", tag=f"s_{nm}")
            mul(s, eg_ap, y)
            return s

        # ================= layer 1 prep (natural layout [512,8]) =========
        # v1n = e1n*md + w1n*mo via host mask constants: 3 big DVE ops
        v1n = wgt.tile([P, 32], f32, name="v1n", tag="v1n")
        n1 = wgt.tile([P, 4], f32, name="n1", tag="n1")
        vT1 = wgt.tile([8, H], fp16, name="vT1", tag="vT1")
        with tc.high_priority():
            v1a = scr.tile([P, 32], f32, name="v1a", tag="v1a")
            mul(v1a, e1n, sm("md1n"))
            v1b = scr.tile([P, 32], f32, name="v1b", tag="v1b")
            mul(v1b, w1n, sm("mo1n"))
            tt(v1n, v1a, v1b, op=ALU.add)
            for c in range(4):
                sq1 = scr.tile([P, 8], f32, name=f"sq1_{c}", tag="sq1")
                stt(sq1, v1n[:, 8 * c:8 * c + 8], 0, v1n[:, 8 * c:8 * c + 8],
                    op0=ALU.bypass, op1=ALU.mult, accum_out=n1[:, c:c + 1])
            s1 = make_scale(n1, eg[1], (P, 4), "l1")
            # vT1 [8, 512] via PE transposes of v1n chunks
            for c in range(4):
                pt = pn.tile([8, P], f32, name=f"pt1_{c}", tag="pn")
                nc.tensor.transpose(pt, v1n[:, 8 * c:8 * c + 8], ident)
                cp(vT1[:, P * c:P * c + P], pt)

        # ================= layer 1 batch ==================================
        h1 = wgt.tile([P, 4 * BC], fp16, name="h1", tag="h1")
        with tc.high_priority():
            for c in range(4):
                pzc = pz.tile([P, BC], f32, name=f"pz1_{c}", tag="pz")
                mm(pzc, vT1[:, P * c:P * c + P], xT)
                act(h1[:, BC * c:BC * c + BC], pzc, E.Tanh,
                    bias=sm("b1")[:, c:c + 1], scale=s1[:, c:c + 1])
        hq1 = scr.tile([P, 4 * BC], fp16, name="hq1", tag="hq")
        mul(hq1, h1, h1)
        sc1 = scr.tile([P, 4 * BC], fp16, name="sc1", tag="sech2")
        ts(sc1, hq1, -1.0, 1.0, op0=ALU.mult, op1=ALU.add)
        G1 = wgt.tile([P, 4 * BC], bf16, name="G1", tag="G1")
        for c in range(4):
            ts(G1[:, BC * c:BC * c + BC], sc1[:, BC * c:BC * c + BC],
               e1d[:, c:c + 1], None, op0=ALU.mult)

        # ================= layer 2/3 prep ================================
        def big_prep(l, s_prev):
            vt = vt_t[l]
            for k in range(4):
                F = H * k
                dA = (slice(0, 64), slice(F + P * k, F + P * k + 64))
                dB = (slice(64, 128), slice(F + P * k + 64, F + P * k + P))
                act(vt[dA], vt[dA], E.Exp)                  # in-place exp(W)
                act(vt[dB], vt[dB], E.Exp)
                nc.vector.memset(vt[64:128, F + P * k:F + P * k + 64], 0.0)
            # vsq = vt^2 over each chunk's live window (ACT Square, same table)
            vsq = stg.tile([P, 4 * H], fp16, name=f"vsq{l}", tag="vsq")
            for k in range(4):
                F = H * k
                act(vsq[:, F + P * k:F + H], vt[:, F + P * k:F + H], E.Square)
            # norm2 row: ones-stationary window matmuls (trivial LDWEIGHTS)
            nrow = pn.tile([2, H], f32, name=f"nrow{l}", tag="pn")
            for k in range(4):
                mm(nrow[:, P * k:H], ones2, vsq[:, H * k + P * k:H * k + H],
                   start=(k == 0), stop=(k == 3), skip_group_check=True)
            nrs = scr.tile([1, H], f32, name=f"nrs{l}", tag="nrs")
            act(nrs, nrow[0:1, :], E.Copy)
            ncol = pn.tile([P, 4], f32, name=f"ncol{l}", tag="pn")
            for c in range(4):
                mm(ncol[:, c:c + 1], nrs[0:1, P * c:P * c + P], ones2f[0:1, 0:1])
            s = make_scale(ncol, eg[l], (P, 4), f"l{l}")
            wd = wgt.tile([P, H], bf16, name=f"wd{l}", tag=f"wd{l}")
            for c in range(4):
                Fw = P * c
                ts(wd[:, Fw:Fw + 128], vt[:, H * c + P * c:H * c + P * c + 128],
                   s_prev[:, c:c + 1], None, op0=ALU.mult)
                nc.vector.memset(wd[0:64, Fw + 64:Fw + 128], 0.0)
                nc.vector.memset(wd[64:128, Fw:Fw + 64], 0.0)
            return vt, wd, s

        vt2, wd2, s2 = big_prep(2, s1)

        # ================= layer 2/3 batch ================================
        def big_batch(l, vt, wd, s, h_prev, G_prev):
            hl = wgt.tile([P, 4 * BC], fp16, name=f"h{l}", tag=f"h{l}")
            pfl = pf.tile([P, 4 * BC], f32, name=f"pf{l}", tag="pf")
            for c in range(4):
                pzc = pz.tile([P, BC], f32, name=f"pz{l}_{c}", tag="pz")
                for k in range(c + 1):
                    mm(pzc, vt[:, H * k + P * c:H * k + P * c + P],
                       h_prev[:, BC * k:BC * k + BC],
                       start=(k == 0), stop=(k == c))
                act(hl[:, BC * c:BC * c + BC], pzc, E.Tanh,
                    bias=sm(f"b{l}")[:, c:c + 1], scale=s[:, c:c + 1])
                mm(pfl[:, BC * c:BC * c + BC], wd[:, P * c:P * c + P],
                   G_prev[:, BC * c:BC * c + BC])
            hql = scr.tile([P, 4 * BC], fp16, name=f"hq{l}", tag="hq")
            mul(hql, hl, hl)
            scl = scr.tile([P, 4 * BC], fp16, name=f"sc{l}", tag="sech2")
            ts(scl, hql, -1.0, 1.0, op0=ALU.mult, op1=ALU.add)
            Gl = wgt.tile([P, 4 * BC], bf16, name=f"G{l}", tag=f"G{l}")
            mul(Gl, pfl, scl)
            return hl, Gl

        h2, G2 = big_batch(2, vt2, wd2, s2, h1, G1)

        vt3, wd3, s3 = big_prep(3, s2)

        # ================= layer 4 prep (transposed [512,8]) ==============
        # vt4 = e4t*md + w4t*mo via host masks; vd4 = (vt4*md)*s3[col k]
        vt4 = wgt.tile([P, 32], fp16, name="vt4", tag="vt4")
        v4a = scr.tile([P, 32], f32, name="v4a", tag="v4a")
        mul(v4a, e4t, sm("md4t"))
        v4b = scr.tile([P, 32], f32, name="v4b", tag="v4b")
        mul(v4b, w4t, sm("mo4t"))
        tt(vt4, v4a, v4b, op=ALU.add)
        vsq4 = scr.tile([P, 32], fp16, name="vsq4", tag="vsq4")
        act(vsq4, vt4, E.Square)
        pn4 = pn.tile([8, 2], f32, name="pn4", tag="pn")
        for k in range(4):
            mm(pn4, vsq4[:, 8 * k:8 * k + 8], ones2, start=(k == 0), stop=(k == 3))
        s4 = make_scale(pn4[:, 0:1], eg4, (8, 1), "l4")
        vd4 = wgt.tile([P, 32], bf16, name="vd4", tag="vd4")
        for k in range(4):
            ts(vd4[:, 8 * k:8 * k + 8], v4a[:, 8 * k:8 * k + 8],
               s3[:, k:k + 1], None, op0=ALU.mult)

        h3, G3 = big_batch(3, vt3, wd3, s3, h2, G2)

        # ================= layer 4 batch ==================================
        pz4 = pz.tile([8, BC], f32, name="pz4", tag="pz")
        for k in range(4):
            mm(pz4, vt4[:, 8 * k:8 * k + 8], h3[:, BC * k:BC * k + BC],
               start=(k == 0), stop=(k == 3))
        h4 = wgt.tile([8, BC], f32, name="h4", tag="h4")
        act(h4, pz4, E.Tanh, bias=b4, scale=s4)
        nc.sync.dma_start(t["hT_out"], h4)
        pf4 = pf.tile([8, BC], f32, name="pf4", tag="pf")
        for k in range(4):
            mm(pf4, vd4[:, 8 * k:8 * k + 8], G3[:, BC * k:BC * k + BC],
               start=(k == 0), stop=(k == 3))
        hq4 = scr.tile([8, BC], f32, name="hq4", tag="hq4")
        mul(hq4, h4, h4)
        s24 = scr.tile([8, BC], f32, name="s24", tag="s24")
        ts(s24, hq4, -1.0, 1.0, op0=ALU.mult, op1=ALU.add)
        gt = scr.tile([8, BC], f32, name="gt", tag="gt")
        stt(gt, pf4, s4, s24, op0=ALU.mult, op1=ALU.mult)
        sld = wgt.tile([8, BC], f32, name="sld", tag="sld")
        act(sld, gt, E.Ln)
        nc.sync.dma_start(t["sldT_out"], sld)

    nc.compile()
    return nc


def _host_prep(x, W1, logg1, bias1, W2, logg2, bias2, W3, logg3, bias3,
               W4, logg4, bias4):
    """Pure layout prep (transpose / reshape / gather / masks), no arithmetic."""
    f = np.float32

    def cols(a):          # [512]-ish vector -> [128, 4] column-chunk layout
        return np.ascontiguousarray(np.reshape(a, (4, P)).T).astype(f)

    smalls = np.zeros((P, SMALL_W), f)

    def put(name, arr):
        a, b = _SM[name]
        smalls[:arr.shape[0], a:b] = arr

    def fold(m):          # [512, 8] -> [128, (k x)] with k = row-chunk
        return m.reshape(4, P, 8).transpose(1, 0, 2).reshape(P, 32)

    put("ident", np.eye(P, dtype=f))
    put("w1n", fold(W1))                                   # natural [512,8]
    put("w4t", fold(np.ascontiguousarray(W4.T)))           # [512,8]
    put("w1dg", cols(W1[np.arange(H), np.arange(H) // 64]))
    put("w4dg", cols(W4[np.arange(H) // 64, np.arange(H)]))
    put("lg1", cols(logg1)); put("b1", cols(bias1))
    put("lg2", cols(logg2)); put("b2", cols(bias2))
    put("lg3", cols(logg3)); put("b3", cols(bias3))
    put("lg4c", logg4.reshape(8, 1).astype(f))
    put("b4c", bias4.reshape(8, 1).astype(f))
    # structural masks
    o = np.arange(H)[:, None] // 64
    i1 = np.arange(8)[None, :]
    md1 = (i1 == o).astype(f); mo1 = (i1 < o).astype(f)    # [512, 8] natural
    put("md1n", fold(md1)); put("mo1n", fold(mo1))
    ii = np.arange(H)[:, None] // 64
    o4 = np.arange(8)[None, :]
    md4 = (o4 == ii).astype(f); mo4 = (o4 > ii).astype(f)  # [512, 8] transposed
    put("md4t", fold(md4)); put("mo4t", fold(mo4))

    w2T = np.ascontiguousarray(W2.T).astype(f)
    w3T = np.ascontiguousarray(W3.T).astype(f)
    xT = np.ascontiguousarray(x.T).astype(f)          # [8, 2048]
    return xT, w2T, w3T, smalls


def kernel(**inputs):
    global LAST_RESULTS
    from concourse.bass_utils import run_bass_kernel_spmd

    xT, w2T, w3T, smalls = _host_prep(**{k: np.asarray(v) for k, v in inputs.items()})

    if "nc" not in _CACHE:
        _CACHE["nc"] = _build()
    nc = _CACHE["nc"]

    in_maps = []
    for c in range(NCORE):
        in_maps.append({
            "xT": np.ascontiguousarray(xT[:, BC * c:BC * (c + 1)]),
            "w2T": w2T, "w3T": w3T, "smalls": smalls,
        })
    res = run_bass_kernel_spmd(nc, in_maps, core_ids=list(range(NCORE)),
                               trace=TRACE)
    LAST_RESULTS = res

    B = BC * NCORE
    h = np.empty((B, 8), np.float32)
    sld = np.empty((B, 8), np.float32)
    for c, r in enumerate(res.results):
        h[BC * c:BC * (c + 1)] = r["hT_out"].T
        sld[BC * c:BC * (c + 1)] = r["sldT_out"].T
    return h, sld

